# revision 1
# baseline (speedup 1.0000x reference)
"""Trainium2 Bass kernel for nn_Conv4Pim_group_split_v2 (dense CNN, PIM-style
group-split quantized conv).

Reference computation (B=32, IC=256, H=W=32, OC=256, GROUPS=4, K=3, pad=1):
  for each branch (p: relu(W), n: relu(-W)) with scales (s_w, s_ps[4]):
    w_int = round(clip(relu_w / s_w, 0, 15));  w_arr = (w_int mod 4) * s_w
    conv  = conv2d(x, w_arr)                        # [B, 4*256, 32, 32]
    per group g: q_g = round(clip(conv_g / s_ps[g], -128, 127)) * s_ps[g]
    branch_out = sum_g q_g                           # [B, 256, 32, 32]
  out = branch_p - branch_n

Kernel strategy:
  - Data-parallel over batch: 8 cores x 4 images, no collectives.
  - Weight quantization done host-side (tiny); device weights are the
    INTEGER values {0,1,2,3} stored in bf16 (exact). The weight scale is
    folded into the psum-quantizer scale alpha = s_w / s_ps.
  - x is split host-side into bf16 hi + lo (x ~= hi + lo, ~16-bit mantissa)
    and the conv runs as 2 accumulating bf16 matmul passes -> near-fp32
    conv accuracy, which matters because the psum quantizer rounds.
  - Conv = 9-offset (3x3) x 2 ic-tile x 2 (hi/lo) = 36 accumulated matmuls
    of [K=128, M=128] x [K=128, N=512] per psum tile, reading a padded
    [128, 34, 34] image held in SBUF.
  - Psum quantize on ACT+DVE: t = psum * alpha (ACT); round via the
    +/- 1.5*2^23 magic trick (DVE, exact RNE like jnp.round); clip to
    [-128,127] (DVE); multiply by +/-s_ps and accumulate group sums (DVE).
"""

import time

import numpy as np
import ml_dtypes
from contextlib import ExitStack

import concourse.bass as bass
import concourse.tile as tile
from concourse import bacc, mybir
from concourse.bass_utils import run_bass_kernel_spmd

dt = mybir.dt
Alu = mybir.AluOpType
AF = mybir.ActivationFunctionType

N_CORES = 8
B, IC, H, W = 32, 256, 32, 32
OC, KS, GROUPS = 256, 3, 4
BPC = B // N_CORES          # batches per core
HP, WP = H + 2, W + 2       # padded image
N_OCT = 16                  # 2048 conv output channels / 128
ROWS_PER_NT = 16            # output rows per psum tile (16*32 = 512 = N)
MAGIC = float(3 * 2**22)    # 1.5*2^23: fp32 RNE rounding constant

_CACHE: dict = {}


def _build_body(ctx: ExitStack, tc, xins, wq, sc, out, n_batches: int,
                n_oct: int, n_iters: int = 1, mode: str = "bf16x2"):
    """Emit the per-core program.

    mode "bf16x2": xins = (xh, xl) bf16 hi/lo DRAM pair, wq bf16.
    mode "fp32r":  xins = (xf,) float32r DRAM, wq float32r, single pass.
    wq:  [128, n_oct*2*9*128] (integer weights, icp-partition)
    sc:  [128, 16] f32 DRAM (col j: alpha_j, col 8+j: beta_j)
    out: [n_batches, 256, 1024] f32 DRAM
    """
    nc = tc.nc
    n_j = n_oct // 2          # number of (branch,group) psum slabs
    n_tout = 2                # output oc tiles (256 oc)
    n_hl = 2 if mode == "bf16x2" else 1
    xdt = dt.bfloat16 if mode == "bf16x2" else dt.float32
    fp32r = mode == "fp32r"
    n_mm = 2 * KS * KS * n_hl

    wpool = ctx.enter_context(tc.tile_pool(name="w", bufs=1))
    spool = ctx.enter_context(tc.tile_pool(name="s", bufs=1))
    xpool = ctx.enter_context(tc.tile_pool(name="x", bufs=2))
    ppool = ctx.enter_context(tc.tile_pool(name="ps", bufs=8, space="PSUM"))
    tpool = ctx.enter_context(tc.tile_pool(name="t", bufs=3))
    apool = ctx.enter_context(tc.tile_pool(name="a", bufs=3))

    sct = spool.tile([128, 16], dt.float32, name="sct")
    nc.sync.dma_start(sct[:], sc[:])

    wdt = dt.float32r if fp32r else xdt
    wt = wpool.tile([128, n_oct * 2 * 9 * 128], wdt, name="wt")
    chunk = 2 * 9 * 128
    for ot in range(n_oct):
        nc.sync.dma_start(wt[:, ot * chunk:(ot + 1) * chunk],
                          wq[:, ot * chunk:(ot + 1) * chunk])

    loop_ctx = tc.For_i(0, n_iters, 1) if n_iters > 1 else None
    if loop_ctx is not None:
        ctx.enter_context(loop_ctx)

    for b in range(n_batches):
        xt = {}
        for ict in range(2):
            for hl in range(n_hl):
                tile_dt = dt.float32r if fp32r else xdt
                t = xpool.tile([128, HP, WP], tile_dt,
                               name=f"xp{ict}{hl}", tag=f"xp{ict}{hl}")
                if fp32r:
                    nc.gpsimd.memset(t.bitcast(dt.uint32), 0)
                else:
                    nc.gpsimd.memset(t[:], 0.0)
                nc.sync.dma_start(t[:, 1:H + 1, 1:W + 1], xins[hl][b, ict])
                xt[ict, hl] = t

        for nt in range(H // ROWS_PER_NT):
            y0 = nt * ROWS_PER_NT
            for tout in range(n_tout):
                acc = apool.tile([128, 512], dt.float32, name="acc", tag="acc")
                for j in range(n_j):
                    ot = 2 * j + tout
                    ps = ppool.tile([128, 512], dt.float32, name="ps", tag="ps")
                    mm = 0
                    for ict in range(2):
                        for ky in range(KS):
                            for kx in range(KS):
                                for hl in range(n_hl):
                                    base = (((ot * 2 + ict) * 3 + ky) * 3 + kx) * 128
                                    lhsT = wt[:, base:base + 128]
                                    rhs = xt[ict, hl][:, y0 + ky:y0 + ky + ROWS_PER_NT,
                                                      kx:kx + W]
                                    nc.tensor.matmul(ps[:], lhsT, rhs,
                                                     start=(mm == 0),
                                                     stop=(mm == n_mm - 1))
                                    mm += 1
                    # quantize: round(clip(ps*alpha, -128, 127)) * beta, accumulate.
                    # Round via the 1.5*2^23 magic constant: ACT computes
                    # ps*alpha + MAGIC (fp32 -> forced RNE to integer), DVE
                    # subtracts it back, then clip and scale by +/-s_ps.
                    t1 = tpool.tile([128, 512], dt.float32, name="t1", tag="t1")
                    nc.scalar.activation(t1[:], ps[:], AF.Copy,
                                         scale=sct[:, j:j + 1], bias=MAGIC)
                    t2 = tpool.tile([128, 512], dt.float32, name="t2", tag="t2")
                    nc.vector.tensor_scalar(t2[:], t1[:], MAGIC, -128.0,
                                            Alu.subtract, Alu.max)
                    if j == 0:
                        nc.vector.tensor_scalar(acc[:], t2[:], 127.0,
                                                sct[:, 8 + j:9 + j],
                                                Alu.min, Alu.mult)
                    else:
                        t3 = tpool.tile([128, 512], dt.float32, name="t3", tag="t3")
                        nc.vector.tensor_scalar(t3[:], t2[:], 127.0,
                                                sct[:, 8 + j:9 + j],
                                                Alu.min, Alu.mult)
                        nc.vector.tensor_add(acc[:], acc[:], t3[:])
                nc.sync.dma_start(
                    out[b, 128 * tout:128 * (tout + 1), 512 * nt:512 * (nt + 1)],
                    acc[:])


def build_program(n_batches: int = BPC, n_oct: int = N_OCT, n_iters: int = 1,
                  mode: str = "bf16x2"):
    nc = bacc.Bacc("TRN2", target_bir_lowering=False, debug=False,
                   enable_asserts=False, num_devices=N_CORES)
    xdt = dt.bfloat16 if mode == "bf16x2" else dt.float32r
    if mode == "bf16x2":
        xins = (nc.dram_tensor("xh", [n_batches, 2, 128, H, W], xdt,
                               kind="ExternalInput").ap(),
                nc.dram_tensor("xl", [n_batches, 2, 128, H, W], xdt,
                               kind="ExternalInput").ap())
    else:
        xins = (nc.dram_tensor("xf", [n_batches, 2, 128, H, W], xdt,
                               kind="ExternalInput").ap(),)
    wq = nc.dram_tensor("wq", [128, n_oct * 2 * 9 * 128], xdt,
                        kind="ExternalInput").ap()
    sc = nc.dram_tensor("sc", [128, 16], dt.float32,
                        kind="ExternalInput").ap()
    out = nc.dram_tensor("out", [n_batches, 256, H * W], dt.float32,
                         kind="ExternalOutput").ap()
    with tile.TileContext(nc) as tc, ExitStack() as ctx:
        _build_body(ctx, tc, xins, wq, sc, out, n_batches, n_oct, n_iters, mode)
    nc.compile()
    return nc


def _quant_weights(weight: np.ndarray, s_w: np.float32) -> np.ndarray:
    """(round(clip(relu_w / s_w, 0, 15)) mod 4) as float32 integers."""
    w = weight.astype(np.float32)
    w_int = np.round(np.clip(w / np.float32(s_w), np.float32(0.0),
                             np.float32(15.0)))
    return np.mod(w_int, np.float32(4.0))


def prep_inputs(x, weight, s_w_p, s_w_n, s_ps_p, s_ps_n, mode: str = "bf16x2"):
    """Host-side prep: returns (in_maps list of 8 dicts)."""
    x = np.asarray(x, dtype=np.float32)
    weight = np.asarray(weight, dtype=np.float32)
    s_w_p = np.float32(np.asarray(s_w_p).reshape(-1)[0])
    s_w_n = np.float32(np.asarray(s_w_n).reshape(-1)[0])
    s_ps_p = np.asarray(s_ps_p, dtype=np.float32).reshape(GROUPS)
    s_ps_n = np.asarray(s_ps_n, dtype=np.float32).reshape(GROUPS)

    wq_p = _quant_weights(np.maximum(weight, 0.0), s_w_p)
    wq_n = _quant_weights(np.maximum(-weight, 0.0), s_w_n)
    w_all = np.concatenate([wq_p, wq_n], axis=0)        # [2048, 256, 3, 3]
    wdt = ml_dtypes.bfloat16 if mode == "bf16x2" else np.float32
    # -> wsb[icp, ot, ict, ky, kx, oci]
    wsb = (w_all.reshape(N_OCT, 128, 2, 128, KS, KS)
           .transpose(3, 0, 2, 4, 5, 1)
           .reshape(128, N_OCT * 2 * 9 * 128)
           .astype(wdt))

    # scales table: col j = alpha_j = s_w/s_ps_j ; col 8+j = beta_j = +/-s_ps_j
    alpha = np.concatenate([s_w_p / s_ps_p, s_w_n / s_ps_n]).astype(np.float32)
    beta = np.concatenate([s_ps_p, -s_ps_n]).astype(np.float32)
    row = np.zeros(16, dtype=np.float32)
    row[0:8] = alpha
    row[8:16] = beta
    sc_np = np.ascontiguousarray(np.tile(row, (128, 1)))

    xs = x.reshape(B, 2, 128, H, W)
    if mode == "bf16x2":
        x_hi = xs.astype(ml_dtypes.bfloat16)
        x_lo = (xs - x_hi.astype(np.float32)).astype(ml_dtypes.bfloat16)
    else:
        x_hi = xs  # float32, fed directly as float32r
        x_lo = None

    in_maps = []
    for c in range(N_CORES):
        sl = slice(c * BPC, (c + 1) * BPC)
        m = {"wq": wsb, "sc": sc_np}
        if mode == "bf16x2":
            m["xh"] = np.ascontiguousarray(x_hi[sl])
            m["xl"] = np.ascontiguousarray(x_lo[sl])
        else:
            m["xf"] = np.ascontiguousarray(x_hi[sl])
        in_maps.append(m)
    return in_maps


MODE = "fp32r"


def kernel(x, weight, s_w_p, s_w_n, s_ps_p, s_ps_n):
    if "nc" not in _CACHE:
        _CACHE["nc"] = build_program(mode=MODE)
    nc = _CACHE["nc"]
    in_maps = prep_inputs(x, weight, s_w_p, s_w_n, s_ps_p, s_ps_n, mode=MODE)
    res = run_bass_kernel_spmd(nc, in_maps, core_ids=list(range(N_CORES)))
    outs = [res.results[c]["out"] for c in range(N_CORES)]
    full = np.concatenate(outs, axis=0).reshape(B, OC, H, W)
    return full.astype(np.float32)


# ---------------------------------------------------------------------------
# Timing helper (not used by the grading harness; mirrors
# bass2jax.run_bass_via_pjrt's multi-core path but keeps the jitted callable
# so repeated executions can be timed without retrace overhead).
# ---------------------------------------------------------------------------

def _make_runner(nc):
    import jax
    from jax.experimental.shard_map import shard_map
    from jax.sharding import Mesh, PartitionSpec
    from concourse import bass2jax

    bass2jax.install_neuronx_cc_hook()
    partition_name = (nc.partition_id_tensor.name
                      if nc.partition_id_tensor else None)
    in_names, out_names, out_avals = [], [], []
    for alloc in nc.m.functions[0].allocations:
        if not isinstance(alloc, mybir.MemoryLocationSet):
            continue
        name = alloc.memorylocations[0].name
        if alloc.kind == "ExternalInput":
            if name != partition_name:
                in_names.append(name)
        elif alloc.kind == "ExternalOutput":
            out_names.append(name)
            out_avals.append(jax.core.ShapedArray(tuple(alloc.tensor_shape),
                                                  mybir.dt.np(alloc.dtype)))
    n_params = len(in_names)
    all_names = list(in_names) + list(out_names)
    if partition_name is not None:
        all_names.append(partition_name)

    def _body(*args):
        operands = list(args)
        if partition_name is not None:
            operands.append(bass2jax.partition_id_tensor())
        outs = bass2jax._bass_exec_p.bind(
            *operands,
            out_avals=tuple(out_avals),
            in_names=tuple(all_names),
            out_names=tuple(out_names),
            lowering_input_output_aliases=(),
            sim_require_finite=False,
            sim_require_nnan=False,
            nc=nc,
        )
        return tuple(outs)

    devices = jax.devices()[:N_CORES]
    mesh = Mesh(np.asarray(devices), ("core",))
    n_outs = len(out_names)
    in_specs = (PartitionSpec("core"),) * (n_params + n_outs)
    out_specs = (PartitionSpec("core"),) * n_outs
    donate = tuple(range(n_params, n_params + n_outs))
    sharded = jax.jit(
        shard_map(_body, mesh=mesh, in_specs=in_specs, out_specs=out_specs,
                  check_rep=False),
        donate_argnums=donate, keep_unused=True)
    return sharded, in_names, out_names, out_avals


def time_device(inputs, iters: int = 10):
    """Min wall time per execution of the staged, pre-jitted program."""
    import jax
    if "nc" not in _CACHE:
        _CACHE["nc"] = build_program(mode=MODE)
    nc = _CACHE["nc"]
    in_maps = prep_inputs(**inputs, mode=MODE)
    sharded, in_names, out_names, out_avals = _make_runner(nc)
    concat_in = [np.concatenate([in_maps[c][n] for c in range(N_CORES)], axis=0)
                 for n in in_names]
    dev_in = [jax.device_put(a) for a in concat_in]
    zeros = [np.zeros((N_CORES * a.shape[0], *a.shape[1:]), a.dtype)
             for a in out_avals]
    # warmup + correctness of path
    out = sharded(*dev_in, *[jax.device_put(z) for z in zeros])
    jax.block_until_ready(out)
    times = []
    for _ in range(iters):
        zdev = [jax.device_put(z) for z in zeros]
        jax.block_until_ready(zdev)
        t0 = time.monotonic()
        out = sharded(*dev_in, *zdev)
        jax.block_until_ready(out)
        times.append(time.monotonic() - t0)
    return min(times) * 1e9



# revision 2
# speedup vs baseline: 1.0424x; 1.0424x over previous
"""Trainium2 Bass kernel for nn_Conv4Pim_group_split_v2 (dense CNN, PIM-style
group-split quantized conv).

Reference computation (B=32, IC=256, H=W=32, OC=256, GROUPS=4, K=3, pad=1):
  for each branch (p: relu(W), n: relu(-W)) with scales (s_w, s_ps[4]):
    w_int = round(clip(relu_w / s_w, 0, 15));  w_arr = (w_int mod 4) * s_w
    conv  = conv2d(x, w_arr)                        # [B, 4*256, 32, 32]
    per group g: q_g = round(clip(conv_g / s_ps[g], -128, 127)) * s_ps[g]
    branch_out = sum_g q_g                           # [B, 256, 32, 32]
  out = branch_p - branch_n

Kernel strategy (mode "fp8dr", the default):
  - Data-parallel over batch: 8 cores x 4 images, no collectives.
  - Weight quantization done host-side; device weights are the INTEGER
    values {0,1,2,3} stored in fp8 e4m3 (exact).  The weight scale is
    folded into the psum-quantizer scale alpha = s_w / s_ps.
  - x is split host-side into e4m3 hi + lo (x ~= hi + lo, ~9-bit
    mantissa), accumulated in fp32 PSUM -> rel err ~5e-3.
  - Conv uses fp8 DoubleRow matmuls (measured ~132 cyc per K=256xN=512
    vs 599 cyc for an fp32r K=128 matmul): the DR pair dim carries the
    two 128-channel ic tiles, so one psum tile needs 9 offsets x 2
    (hi/lo) = 18 DR matmuls instead of 18 full-rate fp32r matmuls.
  - Psum quantize: ONE ACT instruction per psum tile — the fp32->int8
    output conversion of the ACT engine is round-to-nearest-even with
    saturation to [-128,127], which is exactly round(clip(.,-128,127))
    of the reference (verified on HW incl. tie cases).  Then one DVE
    scalar_tensor_tensor accumulates acc = q * (+/-s_ps) + acc.
"""

import time

import numpy as np
import ml_dtypes
from contextlib import ExitStack

import concourse.bass as bass
import concourse.tile as tile
from concourse import bacc, mybir
from concourse.bass_utils import run_bass_kernel_spmd

dt = mybir.dt
Alu = mybir.AluOpType
AF = mybir.ActivationFunctionType
F8 = ml_dtypes.float8_e4m3

N_CORES = 8
B, IC, H, W = 32, 256, 32, 32
OC, KS, GROUPS = 256, 3, 4
BPC = B // N_CORES          # batches per core
HP, WP = H + 2, W + 2       # padded image
N_OCT = 16                  # 2048 conv output channels / 128
ROWS_PER_NT = 16            # output rows per psum tile (16*32 = 512 = N)
MAGIC = float(3 * 2**22)    # 1.5*2^23: fp32 RNE rounding constant

_CACHE: dict = {}


def _build_body_fp8dr(ctx: ExitStack, tc, xq, wq, sc, out, n_batches: int,
                      n_iters: int = 1):
    """fp8 DoubleRow conv + int8-convert quantizer.

    xq: [n_batches, 2(hl), 2(ict), 128, H, W] f8   (hi/lo split of x)
    wq: [128, 16(ot), 2(ict), 9, 128(oci)] f8      (integer weights)
    sc: [128, 16] f32 (col j: alpha_j, col 8+j: beta_j)
    out: [n_batches, 256, 1024] f32
    """
    nc = tc.nc
    DR = mybir.MatmulPerfMode.DoubleRow

    wpool = ctx.enter_context(tc.tile_pool(name="w", bufs=1))
    spool = ctx.enter_context(tc.tile_pool(name="s", bufs=1))
    xpool = ctx.enter_context(tc.tile_pool(name="x", bufs=2))
    ppool = ctx.enter_context(tc.tile_pool(name="ps", bufs=8, space="PSUM"))
    tpool = ctx.enter_context(tc.tile_pool(name="t", bufs=4))
    apool = ctx.enter_context(tc.tile_pool(name="a", bufs=3))

    sct = spool.tile([128, 16], dt.float32, name="sct")
    nc.sync.dma_start(sct[:], sc[:])

    wt = wpool.tile([128, N_OCT, 2, 9, 128], dt.float8e4, name="wt")
    for ot in range(N_OCT):
        nc.sync.dma_start(wt[:, ot], wq[:, ot])

    loop_ctx = tc.For_i(0, n_iters, 1) if n_iters > 1 else None
    if loop_ctx is not None:
        ctx.enter_context(loop_ctx)

    for b in range(n_batches):
        xts = []
        for hl in range(2):
            t = xpool.tile([128, 2, HP, WP], dt.float8e4,
                           name=f"x{hl}", tag=f"x{hl}")
            nc.gpsimd.memset(t[:], 0.0)
            for ict in range(2):
                nc.sync.dma_start(t[:, ict, 1:H + 1, 1:W + 1],
                                  xq[b, hl, ict])
            xts.append(t)

        for nt in range(H // ROWS_PER_NT):
            y0 = nt * ROWS_PER_NT
            for tout in range(2):
                acc = apool.tile([128, 512], dt.float32, name="acc", tag="acc")
                for j in range(8):
                    ot = 2 * j + tout
                    ps = ppool.tile([128, 512], dt.float32, name="ps", tag="ps")
                    mm = 0
                    for ky in range(KS):
                        for kx in range(KS):
                            for hl in range(2):
                                lhsT = wt[:, ot, :, ky * KS + kx, :]
                                rhs = xts[hl][:, :, y0 + ky:y0 + ky + ROWS_PER_NT,
                                              kx:kx + W]
                                nc.tensor.matmul(ps[:], lhsT, rhs,
                                                 start=(mm == 0),
                                                 stop=(mm == 17),
                                                 perf_mode=DR)
                                mm += 1
                    # round(clip(ps*alpha, -128, 127)) via the ACT engine's
                    # saturating RNE fp32->int8 output conversion.
                    qt = tpool.tile([128, 512], dt.int8, name="qt", tag="qt")
                    nc.scalar.activation(qt[:], ps[:], AF.Copy,
                                         scale=sct[:, j:j + 1], bias=0.0)
                    if j == 0:
                        nc.vector.tensor_scalar_mul(acc[:], qt[:],
                                                    sct[:, 8:9])
                    else:
                        nc.vector.scalar_tensor_tensor(acc[:], qt[:],
                                                       sct[:, 8 + j:9 + j],
                                                       acc[:],
                                                       Alu.mult, Alu.add)
                nc.sync.dma_start(
                    out[b, 128 * tout:128 * (tout + 1), 512 * nt:512 * (nt + 1)],
                    acc[:])


def _build_body(ctx: ExitStack, tc, xins, wq, sc, out, n_batches: int,
                n_oct: int, n_iters: int = 1, mode: str = "bf16x2"):
    """Legacy per-core program (modes bf16x2 / fp32r)."""
    nc = tc.nc
    n_j = n_oct // 2          # number of (branch,group) psum slabs
    n_tout = 2                # output oc tiles (256 oc)
    n_hl = 2 if mode == "bf16x2" else 1
    xdt = dt.bfloat16 if mode == "bf16x2" else dt.float32
    fp32r = mode == "fp32r"
    n_mm = 2 * KS * KS * n_hl

    wpool = ctx.enter_context(tc.tile_pool(name="w", bufs=1))
    spool = ctx.enter_context(tc.tile_pool(name="s", bufs=1))
    xpool = ctx.enter_context(tc.tile_pool(name="x", bufs=2))
    ppool = ctx.enter_context(tc.tile_pool(name="ps", bufs=8, space="PSUM"))
    tpool = ctx.enter_context(tc.tile_pool(name="t", bufs=3))
    apool = ctx.enter_context(tc.tile_pool(name="a", bufs=3))

    sct = spool.tile([128, 16], dt.float32, name="sct")
    nc.sync.dma_start(sct[:], sc[:])

    wdt = dt.float32r if fp32r else xdt
    wt = wpool.tile([128, n_oct * 2 * 9 * 128], wdt, name="wt")
    chunk = 2 * 9 * 128
    for ot in range(n_oct):
        nc.sync.dma_start(wt[:, ot * chunk:(ot + 1) * chunk],
                          wq[:, ot * chunk:(ot + 1) * chunk])

    loop_ctx = tc.For_i(0, n_iters, 1) if n_iters > 1 else None
    if loop_ctx is not None:
        ctx.enter_context(loop_ctx)

    for b in range(n_batches):
        xt = {}
        for ict in range(2):
            for hl in range(n_hl):
                tile_dt = dt.float32r if fp32r else xdt
                t = xpool.tile([128, HP, WP], tile_dt,
                               name=f"xp{ict}{hl}", tag=f"xp{ict}{hl}")
                if fp32r:
                    nc.gpsimd.memset(t.bitcast(dt.uint32), 0)
                else:
                    nc.gpsimd.memset(t[:], 0.0)
                nc.sync.dma_start(t[:, 1:H + 1, 1:W + 1], xins[hl][b, ict])
                xt[ict, hl] = t

        for nt in range(H // ROWS_PER_NT):
            y0 = nt * ROWS_PER_NT
            for tout in range(n_tout):
                acc = apool.tile([128, 512], dt.float32, name="acc", tag="acc")
                for j in range(n_j):
                    ot = 2 * j + tout
                    ps = ppool.tile([128, 512], dt.float32, name="ps", tag="ps")
                    mm = 0
                    for ict in range(2):
                        for ky in range(KS):
                            for kx in range(KS):
                                for hl in range(n_hl):
                                    base = (((ot * 2 + ict) * 3 + ky) * 3 + kx) * 128
                                    lhsT = wt[:, base:base + 128]
                                    rhs = xt[ict, hl][:, y0 + ky:y0 + ky + ROWS_PER_NT,
                                                      kx:kx + W]
                                    nc.tensor.matmul(ps[:], lhsT, rhs,
                                                     start=(mm == 0),
                                                     stop=(mm == n_mm - 1))
                                    mm += 1
                    t1 = tpool.tile([128, 512], dt.float32, name="t1", tag="t1")
                    nc.scalar.activation(t1[:], ps[:], AF.Copy,
                                         scale=sct[:, j:j + 1], bias=MAGIC)
                    t2 = tpool.tile([128, 512], dt.float32, name="t2", tag="t2")
                    nc.vector.tensor_scalar(t2[:], t1[:], MAGIC, -128.0,
                                            Alu.subtract, Alu.max)
                    if j == 0:
                        nc.vector.tensor_scalar(acc[:], t2[:], 127.0,
                                                sct[:, 8 + j:9 + j],
                                                Alu.min, Alu.mult)
                    else:
                        t3 = tpool.tile([128, 512], dt.float32, name="t3", tag="t3")
                        nc.vector.tensor_scalar(t3[:], t2[:], 127.0,
                                                sct[:, 8 + j:9 + j],
                                                Alu.min, Alu.mult)
                        nc.vector.tensor_add(acc[:], acc[:], t3[:])
                nc.sync.dma_start(
                    out[b, 128 * tout:128 * (tout + 1), 512 * nt:512 * (nt + 1)],
                    acc[:])


def build_program(n_batches: int = BPC, n_oct: int = N_OCT, n_iters: int = 1,
                  mode: str = "fp8dr"):
    nc = bacc.Bacc("TRN2", target_bir_lowering=False, debug=False,
                   enable_asserts=False, num_devices=N_CORES)
    out = nc.dram_tensor("out", [n_batches, 256, H * W], dt.float32,
                         kind="ExternalOutput").ap()
    sc = nc.dram_tensor("sc", [128, 16], dt.float32,
                        kind="ExternalInput").ap()
    if mode == "fp8dr":
        xq = nc.dram_tensor("xq", [n_batches, 2, 2, 128, H, W], dt.float8e4,
                            kind="ExternalInput").ap()
        wq = nc.dram_tensor("wq", [128, N_OCT, 2, 9, 128], dt.float8e4,
                            kind="ExternalInput").ap()
        with tile.TileContext(nc) as tc, ExitStack() as ctx:
            _build_body_fp8dr(ctx, tc, xq, wq, sc, out, n_batches, n_iters)
        nc.compile()
        return nc

    xdt = dt.bfloat16 if mode == "bf16x2" else dt.float32r
    if mode == "bf16x2":
        xins = (nc.dram_tensor("xh", [n_batches, 2, 128, H, W], xdt,
                               kind="ExternalInput").ap(),
                nc.dram_tensor("xl", [n_batches, 2, 128, H, W], xdt,
                               kind="ExternalInput").ap())
    else:
        xins = (nc.dram_tensor("xf", [n_batches, 2, 128, H, W], xdt,
                               kind="ExternalInput").ap(),)
    wq = nc.dram_tensor("wq", [128, N_OCT * 2 * 9 * 128], xdt,
                        kind="ExternalInput").ap()
    with tile.TileContext(nc) as tc, ExitStack() as ctx:
        _build_body(ctx, tc, xins, wq, sc, out, n_batches, N_OCT, n_iters, mode)
    nc.compile()
    return nc


def _quant_weights(weight: np.ndarray, s_w: np.float32) -> np.ndarray:
    """(round(clip(relu_w / s_w, 0, 15)) mod 4) as float32 integers."""
    w = weight.astype(np.float32)
    w_int = np.round(np.clip(w / np.float32(s_w), np.float32(0.0),
                             np.float32(15.0)))
    return np.mod(w_int, np.float32(4.0))


def prep_inputs(x, weight, s_w_p, s_w_n, s_ps_p, s_ps_n, mode: str = "fp8dr"):
    """Host-side prep: returns (in_maps list of 8 dicts)."""
    x = np.asarray(x, dtype=np.float32)
    weight = np.asarray(weight, dtype=np.float32)
    s_w_p = np.float32(np.asarray(s_w_p).reshape(-1)[0])
    s_w_n = np.float32(np.asarray(s_w_n).reshape(-1)[0])
    s_ps_p = np.asarray(s_ps_p, dtype=np.float32).reshape(GROUPS)
    s_ps_n = np.asarray(s_ps_n, dtype=np.float32).reshape(GROUPS)

    wq_p = _quant_weights(np.maximum(weight, 0.0), s_w_p)
    wq_n = _quant_weights(np.maximum(-weight, 0.0), s_w_n)
    w_all = np.concatenate([wq_p, wq_n], axis=0)        # [2048, 256, 3, 3]
    if mode == "fp8dr":
        wdt = F8
    elif mode == "bf16x2":
        wdt = ml_dtypes.bfloat16
    else:
        wdt = np.float32
    # -> wsb[icp, ot, ict, ky, kx, oci]
    wsb = (w_all.reshape(N_OCT, 128, 2, 128, KS, KS)
           .transpose(3, 0, 2, 4, 5, 1)
           .reshape(128, N_OCT * 2 * 9 * 128)
           .astype(wdt))
    if mode == "fp8dr":
        wsb = np.ascontiguousarray(wsb.reshape(128, N_OCT, 2, 9, 128))

    # scales table: col j = alpha_j = s_w/s_ps_j ; col 8+j = beta_j = +/-s_ps_j
    alpha = np.concatenate([s_w_p / s_ps_p, s_w_n / s_ps_n]).astype(np.float32)
    beta = np.concatenate([s_ps_p, -s_ps_n]).astype(np.float32)
    row = np.zeros(16, dtype=np.float32)
    row[0:8] = alpha
    row[8:16] = beta
    sc_np = np.ascontiguousarray(np.tile(row, (128, 1)))

    xs = x.reshape(B, 2, 128, H, W)
    if mode == "fp8dr":
        x_hi = xs.astype(F8)
        x_lo = (xs - x_hi.astype(np.float32)).astype(F8)
        # [B, hl, ict, 128, H, W]
        xq_full = np.stack([x_hi, x_lo], axis=1)
    elif mode == "bf16x2":
        x_hi = xs.astype(ml_dtypes.bfloat16)
        x_lo = (xs - x_hi.astype(np.float32)).astype(ml_dtypes.bfloat16)
    else:
        x_hi = xs  # float32, fed directly as float32r
        x_lo = None

    in_maps = []
    for c in range(N_CORES):
        sl = slice(c * BPC, (c + 1) * BPC)
        m = {"wq": wsb, "sc": sc_np}
        if mode == "fp8dr":
            m["xq"] = np.ascontiguousarray(xq_full[sl])
        elif mode == "bf16x2":
            m["xh"] = np.ascontiguousarray(x_hi[sl])
            m["xl"] = np.ascontiguousarray(x_lo[sl])
        else:
            m["xf"] = np.ascontiguousarray(x_hi[sl])
        in_maps.append(m)
    return in_maps


MODE = "fp8dr"


def kernel(x, weight, s_w_p, s_w_n, s_ps_p, s_ps_n):
    if "nc" not in _CACHE:
        _CACHE["nc"] = build_program(mode=MODE)
    nc = _CACHE["nc"]
    in_maps = prep_inputs(x, weight, s_w_p, s_w_n, s_ps_p, s_ps_n, mode=MODE)
    res = run_bass_kernel_spmd(nc, in_maps, core_ids=list(range(N_CORES)))
    outs = [res.results[c]["out"] for c in range(N_CORES)]
    full = np.concatenate(outs, axis=0).reshape(B, OC, H, W)
    return full.astype(np.float32)


# ---------------------------------------------------------------------------
# Timing helper (not used by the grading harness; mirrors
# bass2jax.run_bass_via_pjrt's multi-core path but keeps the jitted callable
# so repeated executions can be timed without retrace overhead).
# ---------------------------------------------------------------------------

def _make_runner(nc):
    import jax
    from jax.experimental.shard_map import shard_map
    from jax.sharding import Mesh, PartitionSpec
    from concourse import bass2jax

    bass2jax.install_neuronx_cc_hook()
    partition_name = (nc.partition_id_tensor.name
                      if nc.partition_id_tensor else None)
    in_names, out_names, out_avals = [], [], []
    for alloc in nc.m.functions[0].allocations:
        if not isinstance(alloc, mybir.MemoryLocationSet):
            continue
        name = alloc.memorylocations[0].name
        if alloc.kind == "ExternalInput":
            if name != partition_name:
                in_names.append(name)
        elif alloc.kind == "ExternalOutput":
            out_names.append(name)
            out_avals.append(jax.core.ShapedArray(tuple(alloc.tensor_shape),
                                                  mybir.dt.np(alloc.dtype)))
    n_params = len(in_names)
    all_names = list(in_names) + list(out_names)
    if partition_name is not None:
        all_names.append(partition_name)

    def _body(*args):
        operands = list(args)
        if partition_name is not None:
            operands.append(bass2jax.partition_id_tensor())
        outs = bass2jax._bass_exec_p.bind(
            *operands,
            out_avals=tuple(out_avals),
            in_names=tuple(all_names),
            out_names=tuple(out_names),
            lowering_input_output_aliases=(),
            sim_require_finite=False,
            sim_require_nnan=False,
            nc=nc,
        )
        return tuple(outs)

    devices = jax.devices()[:N_CORES]
    mesh = Mesh(np.asarray(devices), ("core",))
    n_outs = len(out_names)
    in_specs = (PartitionSpec("core"),) * (n_params + n_outs)
    out_specs = (PartitionSpec("core"),) * n_outs
    donate = tuple(range(n_params, n_params + n_outs))
    sharded = jax.jit(
        shard_map(_body, mesh=mesh, in_specs=in_specs, out_specs=out_specs,
                  check_rep=False),
        donate_argnums=donate, keep_unused=True)
    return sharded, in_names, out_names, out_avals


def time_device(inputs, iters: int = 10):
    """Min wall time per execution of the staged, pre-jitted program."""
    import jax
    if "nc" not in _CACHE:
        _CACHE["nc"] = build_program(mode=MODE)
    nc = _CACHE["nc"]
    in_maps = prep_inputs(**inputs, mode=MODE)
    sharded, in_names, out_names, out_avals = _make_runner(nc)
    concat_in = [np.concatenate([in_maps[c][n] for c in range(N_CORES)], axis=0)
                 for n in in_names]
    dev_in = [jax.device_put(a) for a in concat_in]
    zeros = [np.zeros((N_CORES * a.shape[0], *a.shape[1:]), a.dtype)
             for a in out_avals]
    # warmup + correctness of path
    out = sharded(*dev_in, *[jax.device_put(z) for z in zeros])
    jax.block_until_ready(out)
    times = []
    for _ in range(iters):
        zdev = [jax.device_put(z) for z in zeros]
        jax.block_until_ready(zdev)
        t0 = time.monotonic()
        out = sharded(*dev_in, *zdev)
        jax.block_until_ready(out)
        times.append(time.monotonic() - t0)
    return min(times) * 1e9


# revision 4
# speedup vs baseline: 1.1063x; 1.0613x over previous
"""Trainium2 Bass kernel for nn_Conv4Pim_group_split_v2 (dense CNN, PIM-style
group-split quantized conv).

Reference computation (B=32, IC=256, H=W=32, OC=256, GROUPS=4, K=3, pad=1):
  for each branch (p: relu(W), n: relu(-W)) with scales (s_w, s_ps[4]):
    w_int = round(clip(relu_w / s_w, 0, 15));  w_arr = (w_int mod 4) * s_w
    conv  = conv2d(x, w_arr)                        # [B, 4*256, 32, 32]
    per group g: q_g = round(clip(conv_g / s_ps[g], -128, 127)) * s_ps[g]
    branch_out = sum_g q_g                           # [B, 256, 32, 32]
  out = branch_p - branch_n

Kernel strategy (mode "fp8dr", the default):
  - Data-parallel over batch: 8 cores x 4 images, no collectives.
  - Weight quantization done host-side; device weights are the INTEGER
    values {0,1,2,3} stored in fp8 e4m3 (exact).  The weight scale is
    folded into the psum-quantizer scale alpha = s_w / s_ps.
  - x is split host-side into e4m3 hi + lo (x ~= hi + lo, ~9-bit
    mantissa), accumulated in fp32 PSUM -> rel err ~5e-3.
  - Conv uses fp8 DoubleRow matmuls (measured ~132 cyc per K=256xN=512
    vs 599 cyc for an fp32r K=128 matmul): the DR pair dim carries the
    two 128-channel ic tiles, so one psum tile needs 9 offsets x 2
    (hi/lo) = 18 DR matmuls instead of 18 full-rate fp32r matmuls.
  - Psum quantize: ONE ACT instruction per psum tile — the fp32->int8
    output conversion of the ACT engine is round-to-nearest-even with
    saturation to [-128,127], which is exactly round(clip(.,-128,127))
    of the reference (verified on HW incl. tie cases).  Then one DVE
    scalar_tensor_tensor accumulates acc = q * (+/-s_ps) + acc.
"""

import time

import numpy as np
import ml_dtypes
from contextlib import ExitStack

import concourse.bass as bass
import concourse.tile as tile
from concourse import bacc, mybir
from concourse.bass_utils import run_bass_kernel_spmd

dt = mybir.dt
Alu = mybir.AluOpType
AF = mybir.ActivationFunctionType
F8 = ml_dtypes.float8_e4m3

N_CORES = 8
B, IC, H, W = 32, 256, 32, 32
OC, KS, GROUPS = 256, 3, 4
BPC = B // N_CORES          # batches per core
HP, WP = H + 2, W + 2       # padded image
N_OCT = 16                  # 2048 conv output channels / 128
ROWS_PER_NT = 16            # output rows per psum tile (16*32 = 512 = N)
MAGIC = float(3 * 2**22)    # 1.5*2^23: fp32 RNE rounding constant

_CACHE: dict = {}


def _build_body_fp8dr(ctx: ExitStack, tc, xq, wq, sc, out, n_batches: int,
                      n_iters: int = 1):
    """fp8 DoubleRow conv + int8-convert quantizer.

    xq: [n_batches, 2(hl), 2(ict), 128, H, W] f8   (hi/lo split of x)
    wq: [128, 16(ot), 2(ict), 9, 128(oci)] f8      (integer weights)
    sc: [128, 16] f32 (col j: alpha_j, col 8+j: beta_j)
    out: [n_batches, 256, 1024] f32
    """
    nc = tc.nc
    DR = mybir.MatmulPerfMode.DoubleRow

    wpool = ctx.enter_context(tc.tile_pool(name="w", bufs=1))
    spool = ctx.enter_context(tc.tile_pool(name="s", bufs=1))
    xpool = ctx.enter_context(tc.tile_pool(name="x", bufs=1))
    ppool = ctx.enter_context(tc.tile_pool(name="ps", bufs=8, space="PSUM"))
    tpool = ctx.enter_context(tc.tile_pool(name="t", bufs=6))
    apool = ctx.enter_context(tc.tile_pool(name="a", bufs=4))

    sct = spool.tile([128, 16], dt.float32, name="sct")
    nc.sync.dma_start(sct[:], sc[:])

    wt = wpool.tile([128, N_OCT, 2, 9, 128], dt.float8e4, name="wt")
    for ot in range(N_OCT):
        nc.sync.dma_start(wt[:, ot], wq[:, ot])

    # All batches' fp8 x tiles fit in SBUF (18.5 KB/partition): allocate and
    # zero the padded borders once, DMA the interiors upfront, so the conv
    # loop never waits on x at batch boundaries.
    xtiles = {}
    for b in range(n_batches):
        for hl in range(2):
            t = xpool.tile([128, 2, HP, WP], dt.float8e4, name=f"x{b}{hl}")
            nc.gpsimd.memset(t[:], 0.0)
            xtiles[b, hl] = t

    loop_ctx = tc.For_i(0, n_iters, 1) if n_iters > 1 else None
    if loop_ctx is not None:
        ctx.enter_context(loop_ctx)

    for b in range(n_batches):
        for hl in range(2):
            for ict in range(2):
                nc.sync.dma_start(xtiles[b, hl][:, ict, 1:H + 1, 1:W + 1],
                                  xq[b, hl, ict])

    for b in range(n_batches):
        xts = [xtiles[b, 0], xtiles[b, 1]]
        for nt in range(H // ROWS_PER_NT):
            y0 = nt * ROWS_PER_NT
            for tout in range(2):
                acc = apool.tile([128, 512], dt.float32, name="acc", tag="acc")
                for j in range(8):
                    ot = 2 * j + tout
                    ps = ppool.tile([128, 512], dt.float32, name="ps", tag="ps")
                    mm = 0
                    for ky in range(KS):
                        for kx in range(KS):
                            for hl in range(2):
                                lhsT = wt[:, ot, :, ky * KS + kx, :]
                                rhs = xts[hl][:, :, y0 + ky:y0 + ky + ROWS_PER_NT,
                                              kx:kx + W]
                                nc.tensor.matmul(ps[:], lhsT, rhs,
                                                 start=(mm == 0),
                                                 stop=(mm == 17),
                                                 perf_mode=DR)
                                mm += 1
                    # round(clip(ps*alpha, -128, 127)) via the ACT engine's
                    # saturating RNE fp32->int8 output conversion.
                    qt = tpool.tile([128, 512], dt.int8, name="qt", tag="qt")
                    nc.scalar.activation(qt[:], ps[:], AF.Copy,
                                         scale=sct[:, j:j + 1], bias=0.0)
                    if j == 0:
                        nc.vector.tensor_scalar_mul(acc[:], qt[:],
                                                    sct[:, 8:9])
                    else:
                        nc.vector.scalar_tensor_tensor(acc[:], qt[:],
                                                       sct[:, 8 + j:9 + j],
                                                       acc[:],
                                                       Alu.mult, Alu.add)
                nc.sync.dma_start(
                    out[b, 128 * tout:128 * (tout + 1), 512 * nt:512 * (nt + 1)],
                    acc[:])


def _build_body(ctx: ExitStack, tc, xins, wq, sc, out, n_batches: int,
                n_oct: int, n_iters: int = 1, mode: str = "bf16x2"):
    """Legacy per-core program (modes bf16x2 / fp32r)."""
    nc = tc.nc
    n_j = n_oct // 2          # number of (branch,group) psum slabs
    n_tout = 2                # output oc tiles (256 oc)
    n_hl = 2 if mode == "bf16x2" else 1
    xdt = dt.bfloat16 if mode == "bf16x2" else dt.float32
    fp32r = mode == "fp32r"
    n_mm = 2 * KS * KS * n_hl

    wpool = ctx.enter_context(tc.tile_pool(name="w", bufs=1))
    spool = ctx.enter_context(tc.tile_pool(name="s", bufs=1))
    xpool = ctx.enter_context(tc.tile_pool(name="x", bufs=2))
    ppool = ctx.enter_context(tc.tile_pool(name="ps", bufs=8, space="PSUM"))
    tpool = ctx.enter_context(tc.tile_pool(name="t", bufs=3))
    apool = ctx.enter_context(tc.tile_pool(name="a", bufs=3))

    sct = spool.tile([128, 16], dt.float32, name="sct")
    nc.sync.dma_start(sct[:], sc[:])

    wdt = dt.float32r if fp32r else xdt
    wt = wpool.tile([128, n_oct * 2 * 9 * 128], wdt, name="wt")
    chunk = 2 * 9 * 128
    for ot in range(n_oct):
        nc.sync.dma_start(wt[:, ot * chunk:(ot + 1) * chunk],
                          wq[:, ot * chunk:(ot + 1) * chunk])

    loop_ctx = tc.For_i(0, n_iters, 1) if n_iters > 1 else None
    if loop_ctx is not None:
        ctx.enter_context(loop_ctx)

    for b in range(n_batches):
        xt = {}
        for ict in range(2):
            for hl in range(n_hl):
                tile_dt = dt.float32r if fp32r else xdt
                t = xpool.tile([128, HP, WP], tile_dt,
                               name=f"xp{ict}{hl}", tag=f"xp{ict}{hl}")
                if fp32r:
                    nc.gpsimd.memset(t.bitcast(dt.uint32), 0)
                else:
                    nc.gpsimd.memset(t[:], 0.0)
                nc.sync.dma_start(t[:, 1:H + 1, 1:W + 1], xins[hl][b, ict])
                xt[ict, hl] = t

        for nt in range(H // ROWS_PER_NT):
            y0 = nt * ROWS_PER_NT
            for tout in range(n_tout):
                acc = apool.tile([128, 512], dt.float32, name="acc", tag="acc")
                for j in range(n_j):
                    ot = 2 * j + tout
                    ps = ppool.tile([128, 512], dt.float32, name="ps", tag="ps")
                    mm = 0
                    for ict in range(2):
                        for ky in range(KS):
                            for kx in range(KS):
                                for hl in range(n_hl):
                                    base = (((ot * 2 + ict) * 3 + ky) * 3 + kx) * 128
                                    lhsT = wt[:, base:base + 128]
                                    rhs = xt[ict, hl][:, y0 + ky:y0 + ky + ROWS_PER_NT,
                                                      kx:kx + W]
                                    nc.tensor.matmul(ps[:], lhsT, rhs,
                                                     start=(mm == 0),
                                                     stop=(mm == n_mm - 1))
                                    mm += 1
                    t1 = tpool.tile([128, 512], dt.float32, name="t1", tag="t1")
                    nc.scalar.activation(t1[:], ps[:], AF.Copy,
                                         scale=sct[:, j:j + 1], bias=MAGIC)
                    t2 = tpool.tile([128, 512], dt.float32, name="t2", tag="t2")
                    nc.vector.tensor_scalar(t2[:], t1[:], MAGIC, -128.0,
                                            Alu.subtract, Alu.max)
                    if j == 0:
                        nc.vector.tensor_scalar(acc[:], t2[:], 127.0,
                                                sct[:, 8 + j:9 + j],
                                                Alu.min, Alu.mult)
                    else:
                        t3 = tpool.tile([128, 512], dt.float32, name="t3", tag="t3")
                        nc.vector.tensor_scalar(t3[:], t2[:], 127.0,
                                                sct[:, 8 + j:9 + j],
                                                Alu.min, Alu.mult)
                        nc.vector.tensor_add(acc[:], acc[:], t3[:])
                nc.sync.dma_start(
                    out[b, 128 * tout:128 * (tout + 1), 512 * nt:512 * (nt + 1)],
                    acc[:])


def build_program(n_batches: int = BPC, n_oct: int = N_OCT, n_iters: int = 1,
                  mode: str = "fp8dr"):
    nc = bacc.Bacc("TRN2", target_bir_lowering=False, debug=False,
                   enable_asserts=False, num_devices=N_CORES)
    out = nc.dram_tensor("out", [n_batches, 256, H * W], dt.float32,
                         kind="ExternalOutput").ap()
    sc = nc.dram_tensor("sc", [128, 16], dt.float32,
                        kind="ExternalInput").ap()
    if mode == "fp8dr":
        xq = nc.dram_tensor("xq", [n_batches, 2, 2, 128, H, W], dt.float8e4,
                            kind="ExternalInput").ap()
        wq = nc.dram_tensor("wq", [128, N_OCT, 2, 9, 128], dt.float8e4,
                            kind="ExternalInput").ap()
        with tile.TileContext(nc) as tc, ExitStack() as ctx:
            _build_body_fp8dr(ctx, tc, xq, wq, sc, out, n_batches, n_iters)
        nc.compile()
        return nc

    xdt = dt.bfloat16 if mode == "bf16x2" else dt.float32r
    if mode == "bf16x2":
        xins = (nc.dram_tensor("xh", [n_batches, 2, 128, H, W], xdt,
                               kind="ExternalInput").ap(),
                nc.dram_tensor("xl", [n_batches, 2, 128, H, W], xdt,
                               kind="ExternalInput").ap())
    else:
        xins = (nc.dram_tensor("xf", [n_batches, 2, 128, H, W], xdt,
                               kind="ExternalInput").ap(),)
    wq = nc.dram_tensor("wq", [128, N_OCT * 2 * 9 * 128], xdt,
                        kind="ExternalInput").ap()
    with tile.TileContext(nc) as tc, ExitStack() as ctx:
        _build_body(ctx, tc, xins, wq, sc, out, n_batches, N_OCT, n_iters, mode)
    nc.compile()
    return nc


def _quant_weights(weight: np.ndarray, s_w: np.float32) -> np.ndarray:
    """(round(clip(relu_w / s_w, 0, 15)) mod 4) as float32 integers."""
    w = weight.astype(np.float32)
    w_int = np.round(np.clip(w / np.float32(s_w), np.float32(0.0),
                             np.float32(15.0)))
    return np.mod(w_int, np.float32(4.0))


def prep_inputs(x, weight, s_w_p, s_w_n, s_ps_p, s_ps_n, mode: str = "fp8dr"):
    """Host-side prep: returns (in_maps list of 8 dicts)."""
    x = np.asarray(x, dtype=np.float32)
    weight = np.asarray(weight, dtype=np.float32)
    s_w_p = np.float32(np.asarray(s_w_p).reshape(-1)[0])
    s_w_n = np.float32(np.asarray(s_w_n).reshape(-1)[0])
    s_ps_p = np.asarray(s_ps_p, dtype=np.float32).reshape(GROUPS)
    s_ps_n = np.asarray(s_ps_n, dtype=np.float32).reshape(GROUPS)

    wq_p = _quant_weights(np.maximum(weight, 0.0), s_w_p)
    wq_n = _quant_weights(np.maximum(-weight, 0.0), s_w_n)
    w_all = np.concatenate([wq_p, wq_n], axis=0)        # [2048, 256, 3, 3]
    if mode == "fp8dr":
        wdt = F8
    elif mode == "bf16x2":
        wdt = ml_dtypes.bfloat16
    else:
        wdt = np.float32
    # -> wsb[icp, ot, ict, ky, kx, oci]
    wsb = (w_all.reshape(N_OCT, 128, 2, 128, KS, KS)
           .transpose(3, 0, 2, 4, 5, 1)
           .reshape(128, N_OCT * 2 * 9 * 128)
           .astype(wdt))
    if mode == "fp8dr":
        wsb = np.ascontiguousarray(wsb.reshape(128, N_OCT, 2, 9, 128))

    # scales table: col j = alpha_j = s_w/s_ps_j ; col 8+j = beta_j = +/-s_ps_j
    alpha = np.concatenate([s_w_p / s_ps_p, s_w_n / s_ps_n]).astype(np.float32)
    beta = np.concatenate([s_ps_p, -s_ps_n]).astype(np.float32)
    row = np.zeros(16, dtype=np.float32)
    row[0:8] = alpha
    row[8:16] = beta
    sc_np = np.ascontiguousarray(np.tile(row, (128, 1)))

    xs = x.reshape(B, 2, 128, H, W)
    if mode == "fp8dr":
        x_hi = xs.astype(F8)
        x_lo = (xs - x_hi.astype(np.float32)).astype(F8)
        # [B, hl, ict, 128, H, W]
        xq_full = np.stack([x_hi, x_lo], axis=1)
    elif mode == "bf16x2":
        x_hi = xs.astype(ml_dtypes.bfloat16)
        x_lo = (xs - x_hi.astype(np.float32)).astype(ml_dtypes.bfloat16)
    else:
        x_hi = xs  # float32, fed directly as float32r
        x_lo = None

    in_maps = []
    for c in range(N_CORES):
        sl = slice(c * BPC, (c + 1) * BPC)
        m = {"wq": wsb, "sc": sc_np}
        if mode == "fp8dr":
            m["xq"] = np.ascontiguousarray(xq_full[sl])
        elif mode == "bf16x2":
            m["xh"] = np.ascontiguousarray(x_hi[sl])
            m["xl"] = np.ascontiguousarray(x_lo[sl])
        else:
            m["xf"] = np.ascontiguousarray(x_hi[sl])
        in_maps.append(m)
    return in_maps


MODE = "fp8dr"


def kernel(x, weight, s_w_p, s_w_n, s_ps_p, s_ps_n):
    if "nc" not in _CACHE:
        _CACHE["nc"] = build_program(mode=MODE)
    nc = _CACHE["nc"]
    in_maps = prep_inputs(x, weight, s_w_p, s_w_n, s_ps_p, s_ps_n, mode=MODE)
    res = run_bass_kernel_spmd(nc, in_maps, core_ids=list(range(N_CORES)))
    outs = [res.results[c]["out"] for c in range(N_CORES)]
    full = np.concatenate(outs, axis=0).reshape(B, OC, H, W)
    return full.astype(np.float32)


# ---------------------------------------------------------------------------
# Timing helper (not used by the grading harness; mirrors
# bass2jax.run_bass_via_pjrt's multi-core path but keeps the jitted callable
# so repeated executions can be timed without retrace overhead).
# ---------------------------------------------------------------------------

def _make_runner(nc):
    import jax
    from jax.experimental.shard_map import shard_map
    from jax.sharding import Mesh, PartitionSpec
    from concourse import bass2jax

    bass2jax.install_neuronx_cc_hook()
    partition_name = (nc.partition_id_tensor.name
                      if nc.partition_id_tensor else None)
    in_names, out_names, out_avals = [], [], []
    for alloc in nc.m.functions[0].allocations:
        if not isinstance(alloc, mybir.MemoryLocationSet):
            continue
        name = alloc.memorylocations[0].name
        if alloc.kind == "ExternalInput":
            if name != partition_name:
                in_names.append(name)
        elif alloc.kind == "ExternalOutput":
            out_names.append(name)
            out_avals.append(jax.core.ShapedArray(tuple(alloc.tensor_shape),
                                                  mybir.dt.np(alloc.dtype)))
    n_params = len(in_names)
    all_names = list(in_names) + list(out_names)
    if partition_name is not None:
        all_names.append(partition_name)

    def _body(*args):
        operands = list(args)
        if partition_name is not None:
            operands.append(bass2jax.partition_id_tensor())
        outs = bass2jax._bass_exec_p.bind(
            *operands,
            out_avals=tuple(out_avals),
            in_names=tuple(all_names),
            out_names=tuple(out_names),
            lowering_input_output_aliases=(),
            sim_require_finite=False,
            sim_require_nnan=False,
            nc=nc,
        )
        return tuple(outs)

    devices = jax.devices()[:N_CORES]
    mesh = Mesh(np.asarray(devices), ("core",))
    n_outs = len(out_names)
    in_specs = (PartitionSpec("core"),) * (n_params + n_outs)
    out_specs = (PartitionSpec("core"),) * n_outs
    donate = tuple(range(n_params, n_params + n_outs))
    sharded = jax.jit(
        shard_map(_body, mesh=mesh, in_specs=in_specs, out_specs=out_specs,
                  check_rep=False),
        donate_argnums=donate, keep_unused=True)
    return sharded, in_names, out_names, out_avals


def time_device(inputs, iters: int = 10):
    """Min wall time per execution of the staged, pre-jitted program."""
    import jax
    if "nc" not in _CACHE:
        _CACHE["nc"] = build_program(mode=MODE)
    nc = _CACHE["nc"]
    in_maps = prep_inputs(**inputs, mode=MODE)
    sharded, in_names, out_names, out_avals = _make_runner(nc)
    concat_in = [np.concatenate([in_maps[c][n] for c in range(N_CORES)], axis=0)
                 for n in in_names]
    dev_in = [jax.device_put(a) for a in concat_in]
    zeros = [np.zeros((N_CORES * a.shape[0], *a.shape[1:]), a.dtype)
             for a in out_avals]
    # warmup + correctness of path
    out = sharded(*dev_in, *[jax.device_put(z) for z in zeros])
    jax.block_until_ready(out)
    times = []
    for _ in range(iters):
        zdev = [jax.device_put(z) for z in zeros]
        jax.block_until_ready(zdev)
        t0 = time.monotonic()
        out = sharded(*dev_in, *zdev)
        jax.block_until_ready(out)
        times.append(time.monotonic() - t0)
    return min(times) * 1e9
